# revision 1
# baseline (speedup 1.0000x reference)
"""Trainium2 Bass kernel for nn_EntropywithDis (geo contrastive loss).

Computes, on 8 NeuronCores, the scalar loss of the reference:
  - per-sample candidate pools from a 1M-point gps gallery (gathered
    host-side; the device indirect-DMA ucode takes one descriptor per
    partition per ~1us call, which is far too slow for 10k gathers)
  - haversine distances + (arg)rank -> near/far negative selection
  - the selected negative *coordinates* are produced on-device by the
    same mask-reduce that does the slot matching (no second gather)
  - fourier-feature MLP gps encoder + image projection, cosine logits
  - cross-entropy of the diagonal = mean(diag - logsumexp(row))

Sharding: data-parallel over batch for the mining stage (64 rows/core);
encoder columns per core = [64 gps | 64 pad | 2048 negatives] (2176 =
17*128 columns; the pad block keeps the norm/exp row tiles 128-aligned
and is masked out of the softmax with an exp bias of -90). Logits
[B, B+Q] are sharded column-wise; the host combines the per-core
partial sum-of-exp and diagonal outputs.

Engine schedule (the kernel is latency-bound, so instruction order per
in-order engine queue is load-bearing):
  - sync DGE queue: small inputs, then imgs row tiles, then weights.
    The ACT queue stays empty so activation-table loads never sit
    behind 12MB of weight traffic.
  - mining's SBUF partition-moves ride the DVE DGE queue, which
    self-synchronizes with the DVE compare/select pipeline.
  - PE order: image transposes, image matmuls (lhsT = W_img, so the
    normalized-emb transposes are not needed), mining-tail transpose,
    encoder chunks (gps+pad chunk first; phase B interleaved per
    chunk), then the deferred gps-chunk phase B.
  - image-emb norms run on ACT (square + ones-matmul) and a transposed
    [128,4] Newton rsqrt on DVE after mining drains.
"""

import math

import numpy as np

import concourse.bass as bass
import concourse.mybir as mybir
import concourse.tile as tile
from concourse import bacc
from concourse.bass_utils import run_bass_kernel_spmd
from concourse.masks import make_identity

# ---- problem constants (hardcoded per contract) ----
B, Q, NG = 512, 16384, 1_000_000
D_IMG, E, F_DIM, H_DIM = 2048, 512, 256, 1024
PER = 32          # negatives per sample
POOL = 160        # candidate pool per sample
NEAR_CNT = 48     # pool size - num_far_total
N_FAR = 16
N_CORES = 8
BC = B // N_CORES            # 64 batch rows per core
RC = BC + BC + BC * PER      # 2176 encoder columns (gps + pad + negs)
NEG0 = 2 * BC                # first negative column
DEG = float(np.float32(math.pi / 180.0))
NOISE_STD = float(np.float32(2500.0 / 111320.0))
TWO_PI = float(np.float32(2.0 * math.pi))
PI = float(np.float32(math.pi))
HALF_PI = float(np.float32(math.pi / 2.0))

F32 = mybir.dt.float32
F32R = mybir.dt.float32r
BF16 = mybir.dt.bfloat16
I32 = mybir.dt.int32
AF = mybir.ActivationFunctionType
ALU = mybir.AluOpType
AX = mybir.AxisListType

# encoder column chunks: (start_block, end_block); each block is 64 cols.
# The gps+pad chunk first: it depends only on inputs, so it runs while
# the mining stage is still producing the negative coords.
CHUNKS = [(0, 2), (2, 10), (10, 18), (18, 26), (26, 34)]


def _newton_rsqrt(nc, pool, src_ap, out_ap, shape):
    """out = 1/sqrt(src), elementwise, DVE only (quake seed + 3 Newtons)."""
    p, f = shape
    ivals = pool.tile([p, f], I32, tag="nt_i")
    y = pool.tile([p, f], F32, tag="nt_y")
    qh = pool.tile([p, f], F32, tag="nt_qh")
    t = pool.tile([p, f], F32, tag="nt_t")
    t2 = pool.tile([p, f], F32, tag="nt_t2")
    # i = bits(q) >> 1 ; y0 = bits^-1(magic - i)  == (i * -1 + magic)
    nc.vector.tensor_scalar(
        ivals[:], src_ap.bitcast(I32), 1, None, op0=ALU.arith_shift_right
    )
    nc.vector.tensor_scalar(
        ivals[:], ivals[:], -1, 0x5F3759DF, op0=ALU.mult, op1=ALU.add
    )
    nc.vector.tensor_copy(y[:], ivals[:].bitcast(F32))
    nc.vector.tensor_scalar_mul(qh[:], src_ap, 0.5)
    for _ in range(3):
        nc.vector.tensor_mul(t[:], y[:], y[:])          # y^2
        nc.vector.tensor_mul(t2[:], t[:], qh[:])        # 0.5 q y^2
        nc.vector.tensor_scalar(
            t[:], t2[:], -1.0, 1.5, op0=ALU.mult, op1=ALU.add
        )                                               # 1.5 - 0.5 q y^2
        nc.vector.tensor_mul(y[:], y[:], t[:])
    nc.vector.tensor_copy(out_ap, y[:])


def build_program():
    nc = bacc.Bacc(
        "TRN2", target_bir_lowering=False, debug=False, num_devices=N_CORES
    )

    def din(name, shape, dt=F32):
        return nc.dram_tensor(name, list(shape), dt, kind="ExternalInput").ap()

    def dout(name, shape, dt=F32):
        return nc.dram_tensor(name, list(shape), dt, kind="ExternalOutput").ap()

    imgs_d = din("imgs", [B, D_IMG], F32R)
    w_img_d = din("w_img", [D_IMG, E], F32R)
    w1_d = din("w1", [2 * F_DIM, H_DIM], F32R)
    b1r_d = din("b1r", [128, H_DIM // 128])
    w2_d = din("w2", [H_DIM, E], F32R)
    b2r_d = din("b2r", [128, E // 128])
    freqs_d = din("freqs", [2, F_DIM])
    lgs128_d = din("lgs128", [128, 1])
    pool_gps_d = din("pool_gps", [128, POOL])    # gathered pool coords (split)
    pool_lat_d = din("pool_lat", [128, POOL])    # pool lat, row-dup
    pool_lon_d = din("pool_lon", [128, POOL])    # pool lon, row-dup
    rank_fixd_d = din("rank_fixd", [128, POOL])  # dup-tie fix, row-dup
    gps_dup_d = din("gps_dup", [128, 2])
    gpst_loc_d = din("gpst_loc", [2, BC])
    slot_sp_d = din("slot_sp", [128, PER // 2])  # split slot targets
    noise_sp_d = din("noise_sp", [128, PER])     # (c,s) split layout
    diagmask_d = din("diagmask", [BC, B])
    bexp_d = din("bexp", [128, 1])

    loss_d = dout("loss", [1, 1])
    se_part_d = dout("se_part", [1, B])
    diag_part_d = dout("diag_part", [1, BC])

    with tile.TileContext(nc) as tc:
        with (
            tc.tile_pool(name="consts", bufs=1) as cpool,
            tc.tile_pool(name="psA", bufs=1, space="PSUM") as psA,      # ang M-tiles
            tc.tile_pool(name="psMM", bufs=3, space="PSUM") as psMM,    # big matmuls
            tc.tile_pool(name="psSum", bufs=1, space="PSUM") as psSum,  # sumexp accum
            tc.tile_pool(name="psNq", bufs=1, space="PSUM") as psNq,    # normsq accum
            tc.tile_pool(name="psT", bufs=2, space="PSUM") as psT,      # transposes
        ):
            _ip_cm = tc.tile_pool(name="imgp", bufs=1)
            _iw_cm = tc.tile_pool(name="imgw", bufs=1)
            _mp_cm = tc.tile_pool(name="mine", bufs=1)
            ip = _ip_cm.__enter__()
            iw = _iw_cm.__enter__()
            mp = _mp_cm.__enter__()

            _consts = {}

            def constp(val, p=128):
                if val not in _consts:
                    t = cpool.tile([128, 1], F32, tag=f"const{len(_consts)}")
                    nc.gpsimd.memset(t[:], float(val))
                    _consts[val] = t
                return _consts[val][:p, :]

            # ---------- small inputs first on the sync queue ----------
            pg = mp.tile([128, POOL // 2, 2], F32)
            nc.sync.dma_start(
                pg[:].rearrange("p i c -> p (i c)"), pool_gps_d
            )
            gps_dup = cpool.tile([128, 2], F32)
            nc.sync.dma_start(gps_dup[:], gps_dup_d)
            gpst_loc = cpool.tile([2, BC], F32)
            nc.sync.dma_start(gpst_loc[:], gpst_loc_d)
            freqs = cpool.tile([2, F_DIM], F32)
            nc.sync.dma_start(freqs[:], freqs_d)
            pool_lat = mp.tile([128, POOL], F32)
            nc.sync.dma_start(pool_lat[:], pool_lat_d)
            pool_lon = mp.tile([128, POOL], F32)
            nc.sync.dma_start(pool_lon[:], pool_lon_d)
            rank_fixd = mp.tile([128, POOL], F32)
            nc.sync.dma_start(rank_fixd[:], rank_fixd_d)
            slot_sp = cpool.tile([128, PER // 2], F32)
            nc.sync.dma_start(slot_sp[:], slot_sp_d)
            noise_sp = cpool.tile([128, PER], F32)
            nc.sync.dma_start(noise_sp[:], noise_sp_d)
            diagmask = cpool.tile([BC, B], F32)
            nc.sync.dma_start(diagmask[:], diagmask_d)
            b1r = cpool.tile([128, H_DIM // 128], F32)
            nc.sync.dma_start(b1r[:], b1r_d)
            b2r = cpool.tile([128, E // 128], F32)
            nc.sync.dma_start(b2r[:], b2r_d)
            lgs128 = cpool.tile([128, 1], F32)
            nc.sync.dma_start(lgs128[:], lgs128_d)
            b_exp = cpool.tile([128, 1], F32)
            nc.sync.dma_start(b_exp[:], bexp_d)

            # ---------- Pool-engine constants BEFORE the SWDGE gens ----
            # the library reload that follows the software-DGE descriptor
            # generation barriers on the transfers (~35us of weights), so
            # every gpsimd memset must be emitted before the big loads
            pretouch = cpool.tile([1, 1], F32)
            nc.gpsimd.memset(pretouch[:], 0.0)
            nc.scalar.activation(pretouch[:], pretouch[:], AF.Sin)
            for cv in (HALF_PI, DEG / 2, DEG):
                constp(cv)
            idsrc = cpool.tile([128, 128], F32)
            make_identity(nc, idsrc[:])
            id128 = cpool.tile([128, 128], F32R)
            nc.vector.tensor_copy(id128[:], idsrc[:])
            id1 = idsrc[0:1, 0:1]
            ones32 = cpool.tile([128, 1], F32)
            nc.gpsimd.memset(ones32[:], 1.0)
            ones = cpool.tile([128, 1], F32R)
            nc.vector.tensor_copy(ones[:], ones32[:])
            nq_rt = cpool.tile([128, 20], F32)
            nc.gpsimd.memset(nq_rt[:], 1.0)
            z = cpool.tile([1, 1], F32)
            nc.gpsimd.memset(z[:], 0.0)

            # ---------- big loads via GPSIMD software DGE ----------
            # both hardware DGE queues have tiny ring depth, so parking
            # 12MB of weight loads on either stalls that sequencer for
            # ~35us. The Pool engine is idle; ~1.7us of descriptor
            # generation per load is free concurrency.
            irows = []
            for it in range(B // 128):
                irw = iw.tile([128, D_IMG], F32R, tag=f"irow{it}")
                nc.gpsimd.dma_start(
                    irw[:], imgs_d[it * 128 : (it + 1) * 128, :]
                )
                irows.append(irw)
            w_img = cpool.tile([128, D_IMG // 128, E], F32R)
            for wh in range(2):
                nc.gpsimd.dma_start(
                    w_img[:, wh * 8 : (wh + 1) * 8, :],
                    w_img_d.rearrange("(t p) e -> p t e", p=128)[
                        :, wh * 8 : (wh + 1) * 8, :
                    ],
                )
            w1 = cpool.tile([128, (2 * F_DIM) // 128, H_DIM], F32R)
            nc.gpsimd.dma_start(
                w1[:], w1_d.rearrange("(t p) h -> p t h", p=128)
            )
            w2 = cpool.tile([128, H_DIM // 128, E], F32R)
            nc.gpsimd.dma_start(
                w2[:], w2_d.rearrange("(t p) e -> p t e", p=128)
            )

            # ---------- constants (tiles created in the pre block) ----

            negsT = cpool.tile([32, 128], F32)      # [(c,s), (q,b)]
            coordsT = cpool.tile([2, RC], F32)      # [lat/lon, core columns]
            nc.sync.dma_start(coordsT[:, 0:BC], gpst_loc[:])
            # pad block coords = 0
            nc.vector.tensor_scalar_mul(coordsT[:, BC:NEG0], gpst_loc[:], 0.0)
            img_embT = cpool.tile([128, E // 128, B], F32R)
            rn_bc = cpool.tile([128, B], F32)
            s_rt = cpool.tile([128, 17], F32)       # logit_scale/|g| per col
            diag_sb = cpool.tile([BC, 1], F32)
            se_sb = cpool.tile([1, B], F32)
            diagT = cpool.tile([1, BC], F32)

            # =====================================================
            # Image transposes for the first two row-tiles (their loads
            # land before the mining DVE pipeline needs the PE)
            # =====================================================
            imgsT = ip.tile([128, D_IMG // 128, B], F32R)

            def img_transposes(it):
                irow = irows[it]
                for dt_ in range(D_IMG // 128):
                    pst = psT.tile([128, 128], F32R, tag="tps")
                    nc.tensor.transpose(
                        pst[:],
                        irow[:, dt_ * 128 : (dt_ + 1) * 128],
                        id128[:],
                    )
                    nc.scalar.copy(
                        imgsT[:, dt_, it * 128 : (it + 1) * 128], pst[:]
                    )

            img_transposes(0)
            img_transposes(1)

            # =====================================================
            # Mining head: haversine, rank, select (ACT + DVE)
            # =====================================================
            if True:
                # haversine argument h (monotone in distance), split layout
                lat1d = mp.tile([128, 1], F32)
                nc.vector.tensor_scalar_mul(lat1d[:], gps_dup[:, 0:1], DEG)
                lon1d = mp.tile([128, 1], F32)
                nc.vector.tensor_scalar_mul(lon1d[:], gps_dup[:, 1:2], DEG)
                blat = mp.tile([128, 1], F32)
                nc.vector.tensor_scalar_mul(blat[:], lat1d[:], -0.5)
                blon = mp.tile([128, 1], F32)
                nc.vector.tensor_scalar_mul(blon[:], lon1d[:], -0.5)
                clat1 = mp.tile([128, 1], F32)
                nc.scalar.activation(
                    clat1[:], lat1d[:], AF.Sin, bias=constp(HALF_PI)
                )

                HP = POOL // 2
                lat2 = pg[:, :, 0:1].rearrange("p i one -> p (i one)")
                lon2 = pg[:, :, 1:2].rearrange("p i one -> p (i one)")
                sdlat = mp.tile([128, HP], F32)
                nc.scalar.activation(
                    sdlat[:], lat2, AF.Sin, bias=blat[:], scale=constp(DEG / 2)
                )
                s2dlat = mp.tile([128, HP], F32)
                nc.scalar.activation(s2dlat[:], sdlat[:], AF.Square)
                clat2 = mp.tile([128, HP], F32)
                nc.scalar.activation(
                    clat2[:], lat2, AF.Sin, bias=constp(HALF_PI),
                    scale=constp(DEG)
                )
                cc12 = mp.tile([128, HP], F32)
                nc.vector.tensor_scalar_mul(cc12[:], clat2[:], clat1[:])
                sdlon = mp.tile([128, HP], F32)
                nc.scalar.activation(
                    sdlon[:], lon2, AF.Sin, bias=blon[:], scale=constp(DEG / 2)
                )
                s2dlon = mp.tile([128, HP], F32)
                nc.scalar.activation(s2dlon[:], sdlon[:], AF.Square)
                h2b = mp.tile([128, HP], F32)
                nc.vector.tensor_mul(h2b[:], cc12[:], s2dlon[:])
                nc.vector.tensor_add(h2b[:], h2b[:], s2dlat[:])

                # full per-row copy: h2[(q,b), j] = h[b, j] for all j.
                # The DMA engines are saturated with weight loads in this
                # window, so the partition-crossing halves go via PE
                # transposes (double transpose = partition move) and the
                # same-partition halves via engine copies.
                h2 = mp.tile([128, POOL], F32)
                nc.vector.tensor_copy(h2[0:BC, 0:HP], h2b[0:BC, :])
                nc.vector.tensor_copy(h2[BC:128, HP:POOL], h2b[BC:128, :])
                ps_h = psT.tile([80, 128], F32, tag="tps")
                nc.tensor.transpose(ps_h[:], h2b[:], idsrc[:])
                # stage the transpose column-SWAPPED so one transpose back
                # lands both cross-halves on their home partitions
                h2bT = mp.tile([80, 128], F32)
                nc.scalar.copy(h2bT[:, 0:BC], ps_h[:, BC:128])
                nc.scalar.copy(h2bT[:, BC:128], ps_h[:, 0:BC])
                ps_x = psT.tile([128, 128], F32, tag="tps")
                nc.tensor.transpose(
                    ps_x[:, 0:HP], h2bT[:], idsrc[0:80, 0:80]
                )
                nc.scalar.copy(h2[0:BC, HP:POOL], ps_x[0:BC, 0:HP])
                nc.scalar.copy(h2[BC:128, 0:HP], ps_x[BC:128, 0:HP])

                # rank[i] = #{j : h_j < h_i}; all compares/reduces on DVE
                # (Pool engine has no TensorTensor in the ISA)
                HB = POOL // 16  # 10 i-columns per round
                rank2 = mp.tile([128, POOL // 2], F32)
                for qh in range(8):
                    cmp3 = mp.tile(
                        [128, HB, POOL], BF16, tag=f"cmp3_{qh % 2}"
                    )
                    nc.vector.tensor_tensor(
                        out=cmp3[:],
                        in0=h2[:].unsqueeze(1).to_broadcast([128, HB, POOL]),
                        in1=h2b[:, qh * HB : (qh + 1) * HB]
                        .unsqueeze(2)
                        .to_broadcast([128, HB, POOL]),
                        op=ALU.is_lt,
                    )
                    nc.vector.tensor_reduce(
                        out=rank2[:, qh * HB : (qh + 1) * HB],
                        in_=cmp3[:],
                        axis=AX.X,
                        op=ALU.add,
                    )

                # duplicate full rank rows onto both partition halves:
                # rank_dup[(q,b), 80q'+i] = rank2[(q',b), i]
                rank_dup = mp.tile([128, POOL], F32)
                nc.sync.dma_start(rank_dup[0:BC, 0:HP], rank2[0:BC, :])
                nc.sync.dma_start(rank_dup[0:BC, HP:POOL], rank2[BC:128, :])
                nc.sync.dma_start(rank_dup[BC:128, 0:HP], rank2[0:BC, :])
                nc.sync.dma_start(
                    rank_dup[BC:128, HP:POOL], rank2[BC:128, :]
                )
                rank_a = mp.tile([128, POOL], F32)
                nc.vector.tensor_add(rank_a[:], rank_dup[:], rank_fixd[:])

                # slot match -> negative coords directly: the mask against
                # rank selects the pool point; two mask-reduces yield its
                # lat and lon (no second gather needed). split layout:
                # partition (q,b) handles slots 16q..16q+15.
                HS = PER // 4  # 8 slots per round
                negs_sc = mp.tile([128, PER // 2, 2], F32)  # (slot, lat/lon)
                for sh in range(2):
                    msel = mp.tile([128, HS, POOL], BF16, tag="msel")
                    nc.vector.tensor_tensor(
                        out=msel[:],
                        in0=rank_a[:].unsqueeze(1).to_broadcast([128, HS, POOL]),
                        in1=slot_sp[:, sh * HS : (sh + 1) * HS]
                        .unsqueeze(2)
                        .to_broadcast([128, HS, POOL]),
                        op=ALU.is_equal,
                    )
                    prodm = mp.tile([128, HS, POOL], F32, tag="prodm")
                    for ci, pll in ((0, pool_lat), (1, pool_lon)):
                        nc.vector.tensor_tensor(
                            out=prodm[:],
                            in0=msel[:],
                            in1=pll[:].unsqueeze(1).to_broadcast(
                                [128, HS, POOL]
                            ),
                            op=ALU.mult,
                        )
                        nc.vector.tensor_reduce(
                            out=negs_sc[:, sh * HS : (sh + 1) * HS, ci],
                            in_=prodm[:],
                            axis=AX.X,
                            op=ALU.add,
                        )
                # add noise and reorder to (c, s); noise_sp is host-laid
                # in the same (c,s) layout
                negs2b = mp.tile([128, 2, PER // 2], F32)
                nc.vector.tensor_copy(
                    negs2b[:],
                    negs_sc[:].rearrange("p s c -> p c s"),
                )
                nc.vector.tensor_add(
                    negs2b[:].rearrange("p c s -> p (c s)"),
                    negs2b[:].rearrange("p c s -> p (c s)"),
                    noise_sp[:],
                )

            # =====================================================
            # Image branch (replicated): img_embT = l2norm(imgs@W_img).T
            # =====================================================
            if True:
                img_transposes(2)
                img_transposes(3)
                # embT_raw = W_img^T imgs^T, written straight into img_embT
                # (no transposes of the result needed)
                for et in range(E // 128):
                    pim = psMM.tile([128, B], F32, tag="mm")
                    for kt in range(D_IMG // 128):
                        nc.tensor.matmul(
                            pim[:],
                            lhsT=w_img[:, kt, et * 128 : (et + 1) * 128],
                            rhs=imgsT[:, kt, :],
                            start=(kt == 0),
                            stop=(kt == D_IMG // 128 - 1),
                        )
                    nc.scalar.copy(img_embT[:, et, :], pim[:])
                # per-image norms: column sums of squares via ones-matmul
                pnqi = psNq.tile([1, 512], F32, tag="nq")
                for et in range(E // 128):
                    sqi = ip.tile([128, B], F32R, tag="isq")
                    nc.scalar.activation(sqi[:], img_embT[:, et, :], AF.Square)
                    nc.tensor.matmul(
                        pnqi[:], lhsT=ones[:], rhs=sqi[:],
                        start=(et == 0), stop=(et == E // 128 - 1),
                    )
                imnq = ip.tile([1, B], F32)
                nc.scalar.copy(imnq[:], pnqi[:])   # ACT drains psum early
                # transpose [1,512] -> [128,4], Newton there, transpose back
                imnqT = ip.tile([128, 4], F32)
                for t4 in range(4):
                    pst = psT.tile([128, 128], F32, tag="tps")
                    nc.tensor.transpose(
                        pst[:, 0:1],
                        imnq[0:1, t4 * 128 : (t4 + 1) * 128],
                        id1,
                    )
                    nc.scalar.copy(imnqT[:, t4 : t4 + 1], pst[:, 0:1])
                imrnT = ip.tile([128, 4], F32)
                _newton_rsqrt(nc, ip, imnqT[:], imrnT[:], (128, 4))
                for et in range(E // 128):
                    # partition_broadcast needs a partition-0 source;
                    # the DMA engines are free by the time this runs
                    rn40 = ip.tile([1, 128], F32, tag=f"rn40_{et}")
                    nc.sync.dma_start(rn40[:], imrnT[:, et : et + 1])
                    nc.gpsimd.partition_broadcast(
                        rn_bc[:, et * 128 : (et + 1) * 128], rn40[:]
                    )
                for et in range(E // 128):
                    nc.vector.tensor_mul(
                        img_embT[:, et, :], img_embT[:, et, :], rn_bc[:]
                    )

            # =====================================================
            # Mining tail: transpose + coords assembly (PE after the
            # image matmuls so the tensor engine never stalls on DVE)
            # =====================================================
            if True:
                ps_n = psT.tile([32, 128], F32, tag="tps")
                nc.tensor.transpose(
                    ps_n[:],
                    negs2b[:].rearrange("p c s -> p (c s)"),
                    idsrc[:],
                )
                nc.vector.tensor_copy(negsT[:], ps_n[:])
                # negative column order is (s, q, b) — order is free
                # (logsumexp is order-invariant; noise is pre-paired)
                nc.sync.dma_start(coordsT[0:1, NEG0:RC], negsT[0:16, :])
                nc.sync.dma_start(coordsT[1:2, NEG0:RC], negsT[16:32, :])

            _mp_cm.__exit__(None, None, None)
            _iw_cm.__exit__(None, None, None)
            _ip_cm.__exit__(None, None, None)

            # =====================================================
            # Encoder + logits, chunked over the core's 2176 columns.
            # Phase B (logits+exp+sums) is interleaved per chunk; the
            # gps+pad chunk's B runs last (after img_embT is scaled).
            # =====================================================
            se_ps = psSum.tile([1, B], F32)
            n_rt_total = RC // 128  # 17
            rs_rt = cpool.tile([128, 20], F32)
            rt_global = 0
            with (
                tc.tile_pool(name="gp", bufs=1) as gpool,
                tc.tile_pool(name="enc", bufs=2) as ep,
                tc.tile_pool(name="ench", bufs=2) as eph,
                tc.tile_pool(name="encs", bufs=1) as eps,
            ):
                gcf = gpool.tile([128, E // 128, RC], F32R)  # full g^T

                def emit_B(rt, start, stop):
                    pl = psMM.tile([128, B], F32, tag="mm")
                    for et in range(E // 128):
                        nc.tensor.matmul(
                            pl[:],
                            lhsT=gcf[:, et, rt * 128 : (rt + 1) * 128],
                            rhs=img_embT[:, et, :],
                            start=(et == 0),
                            stop=(et == E // 128 - 1),
                        )
                    if rt == 0:
                        dm = ep.tile([BC, B], F32, tag="dm")
                        nc.vector.tensor_mul(dm[:], pl[0:BC, :], diagmask[:])
                        dv = ep.tile([BC, 1], F32, tag="dv")
                        nc.vector.tensor_reduce(
                            out=dv[:], in_=dm[:], axis=AX.X, op=ALU.add
                        )
                        nc.vector.tensor_scalar_mul(
                            diag_sb[:], dv[:], s_rt[0:BC, 0:1]
                        )
                    expt = ep.tile([128, B], F32R, tag="expt")
                    if rt == 0:
                        nc.scalar.activation(
                            expt[:], pl[:], AF.Exp,
                            scale=s_rt[:, rt : rt + 1], bias=b_exp[:],
                        )
                    else:
                        nc.scalar.activation(
                            expt[:], pl[:], AF.Exp,
                            scale=s_rt[:, rt : rt + 1],
                        )
                    nc.tensor.matmul(
                        se_ps[:], lhsT=ones[:], rhs=expt[:],
                        start=start, stop=stop,
                    )

                for (cb0, cb1) in CHUNKS:
                    cw = 64 * (cb1 - cb0)
                    c0 = cb0 * 64
                    ffc = ep.tile([128, 4, 512], F32R, tag="ffc")
                    for m in range(2):
                        pang = psA.tile([128, 512], F32, tag="ang")
                        nc.tensor.matmul(
                            pang[:, :cw],
                            lhsT=freqs[:, m * 128 : (m + 1) * 128],
                            rhs=coordsT[:, c0 : c0 + cw],
                            start=True,
                            stop=True,
                        )
                        ki = eps.tile([128, 512], I32, tag="ki")
                        nc.vector.tensor_scalar(
                            ki[:, :cw], pang[:, :cw], 1.0 / TWO_PI, None,
                            op0=ALU.mult,
                        )
                        kf = eps.tile([128, 512], F32, tag="kf")
                        nc.vector.tensor_copy(kf[:, :cw], ki[:, :cw])
                        mscr = eps.tile([128, 512], F32, tag="mscr")
                        nc.vector.scalar_tensor_tensor(
                            out=mscr[:, :cw], in0=kf[:, :cw], scalar=-TWO_PI,
                            in1=pang[:, :cw], op0=ALU.mult, op1=ALU.add,
                        )
                        wrap = eps.tile([128, 512], F32, tag="wrap")
                        nc.vector.add_range_wrap(
                            wrap[:, :cw], mscr[:, :cw], 0.0, PI, TWO_PI
                        )
                        nc.scalar.activation(
                            ffc[:, m, :cw], wrap[:, :cw], AF.Sin
                        )
                        wrap2 = eps.tile([128, 512], F32, tag="wrap2")
                        nc.vector.add_range_wrap(
                            wrap2[:, :cw], mscr[:, :cw], HALF_PI, PI, TWO_PI
                        )
                        nc.scalar.activation(
                            ffc[:, 2 + m, :cw], wrap2[:, :cw], AF.Sin
                        )
                    hc = eph.tile([128, H_DIM // 128, 512], F32R, tag="hc")
                    for mh in range(H_DIM // 128):
                        ph = psMM.tile([128, 512], F32, tag="mm")
                        for kt in range(4):
                            nc.tensor.matmul(
                                ph[:, :cw],
                                lhsT=w1[:, kt, mh * 128 : (mh + 1) * 128],
                                rhs=ffc[:, kt, :cw],
                                start=(kt == 0),
                                stop=(kt == 3),
                            )
                        nc.scalar.activation(
                            hc[:, mh, :cw], ph[:, :cw], AF.Relu,
                            bias=b1r[:, mh : mh + 1],
                        )
                    for me in range(E // 128):
                        pg2 = psMM.tile([128, 512], F32, tag="mm")
                        for kt in range(H_DIM // 128):
                            nc.tensor.matmul(
                                pg2[:, :cw],
                                lhsT=w2[:, kt, me * 128 : (me + 1) * 128],
                                rhs=hc[:, kt, :cw],
                                start=(kt == 0),
                                stop=(kt == H_DIM // 128 - 1),
                            )
                        nc.scalar.activation(
                            gcf[:, me, c0 : c0 + cw], pg2[:, :cw],
                            AF.Identity, bias=b2r[:, me : me + 1],
                        )
                    pnq = psNq.tile([1, 512], F32, tag="nq")
                    for me in range(E // 128):
                        sqc = ep.tile([128, 512], F32R, tag="sqc")
                        nc.scalar.activation(
                            sqc[:, :cw], gcf[:, me, c0 : c0 + cw], AF.Square
                        )
                        nc.tensor.matmul(
                            pnq[:, :cw], lhsT=ones[:], rhs=sqc[:, :cw],
                            start=(me == 0), stop=(me == E // 128 - 1),
                        )
                    nq_sb = eps.tile([1, 512], F32, tag="nqsb")
                    nc.vector.tensor_copy(nq_sb[:, :cw], pnq[:, :cw])
                    n_rt = cw // 128
                    for t in range(n_rt):
                        pst = psT.tile([128, 128], F32, tag="tps")
                        nc.tensor.transpose(
                            pst[:, 0:1],
                            nq_sb[0:1, t * 128 : (t + 1) * 128],
                            id1,
                        )
                        nc.vector.tensor_copy(
                            nq_rt[:, rt_global + t : rt_global + t + 1],
                            pst[:, 0:1],
                        )
                    # per-chunk rsqrt of this chunk's norm columns
                    # (fixed width-4 slice so Newton tags keep one shape)
                    nb = min(rt_global, 20 - 4)
                    _newton_rsqrt(
                        nc, ep, nq_rt[:, nb : nb + 4], rs_rt[:, nb : nb + 4],
                        (128, 4),
                    )
                    nc.vector.tensor_scalar_mul(
                        s_rt[:, nb : nb + 4], rs_rt[:, nb : nb + 4], lgs128[:]
                    )
                    # phase B for this chunk (gps+pad chunk deferred)
                    for rt in range(rt_global, rt_global + n_rt):
                        if rt == 0:
                            continue
                        emit_B(rt, start=(rt == 1), stop=False)
                    rt_global += n_rt

                # deferred phase B of the gps+pad tile (needs the scaled
                # img_embT, which is ready once the DVE mining drains)
                emit_B(0, start=False, stop=True)

            # =====================================================
            # Final per-core outputs (host combines across cores)
            # =====================================================
            nc.vector.tensor_copy(se_sb[:], se_ps[:])
            ps_d = psT.tile([1, 64], F32, tag="tps")
            nc.tensor.transpose(ps_d[:], diag_sb[:], idsrc[0:64, 0:64])
            nc.vector.tensor_copy(diagT[:], ps_d[:])

            nc.sync.dma_start(se_part_d, se_sb[:])
            nc.sync.dma_start(diag_part_d, diagT[:])

            nc.sync.dma_start(loss_d, z[:])

    nc.compile()
    return nc


_PROGRAM = None


def _get_program():
    global _PROGRAM
    if _PROGRAM is None:
        _PROGRAM = build_program()
    return _PROGRAM


def make_in_maps(inputs):
    imgs = np.ascontiguousarray(np.asarray(inputs["imgs"], np.float32))
    gps = np.asarray(inputs["gps"], np.float32)
    gallery = np.ascontiguousarray(np.asarray(inputs["gps_gallery"], np.float32))
    w_img = np.ascontiguousarray(np.asarray(inputs["W_img"], np.float32))
    freqs = np.ascontiguousarray(np.asarray(inputs["freqs"], np.float32))
    w1 = np.ascontiguousarray(np.asarray(inputs["W1"], np.float32))
    b1 = np.asarray(inputs["b1"], np.float32)
    w2 = np.ascontiguousarray(np.asarray(inputs["W2"], np.float32))
    b2 = np.asarray(inputs["b2"], np.float32)
    lgs128 = np.full((128, 1), float(np.asarray(inputs["logit_scale"])), np.float32)
    pool_idx = np.asarray(inputs["pool_idx"], np.int32)
    far_sel = np.asarray(inputs["far_sel"], np.int32)
    perm = np.asarray(inputs["perm"], np.int64)

    # deterministic noise constant (jax PRNG, key=1), permuted to neg order.
    # Must be drawn on the CPU backend: the reference runs on cpu-jax and
    # other backends' normal draws are not bit-identical.
    import jax
    import jax.numpy as jnp

    try:
        cpu_dev = jax.local_devices(backend="cpu")[0]
        ctx = jax.default_device(cpu_dev)
    except RuntimeError:
        import contextlib

        ctx = contextlib.nullcontext()
    with ctx:
        noise = np.asarray(
            jax.random.normal(jax.random.key(1), (Q, 2), jnp.float32)
        ) * np.float32(NOISE_STD)
    assert np.array_equal(np.sort(perm), np.arange(Q)), "perm not a permutation"
    noise_p = noise[perm]  # noise seen by negative k

    # stable-rank fix for duplicate pool indices within a row
    eq = pool_idx[:, :, None] == pool_idx[:, None, :]  # [B, i, j]
    tril = np.tril(np.ones((POOL, POOL), bool), -1)[None]  # j < i
    rank_fix = (eq & tril).sum(axis=2).astype(np.float32)

    near_slots = np.tile(np.arange(16, dtype=np.float32), (B, 1))
    slot_full = np.concatenate(
        [near_slots, (NEAR_CNT + far_sel).astype(np.float32)], axis=1
    )

    b1r = np.ascontiguousarray(b1.reshape(H_DIM // 128, 128).T)
    b2r = np.ascontiguousarray(b2.reshape(E // 128, 128).T)

    bexp = np.zeros((128, 1), np.float32)
    bexp[64:, 0] = -90.0

    in_maps = []
    for c in range(N_CORES):
        rows = slice(c * BC, (c + 1) * BC)
        dm = np.zeros((BC, B), np.float32)
        dm[np.arange(BC), c * BC + np.arange(BC)] = 1.0
        ns = noise_p[c * BC * PER : (c + 1) * BC * PER].reshape(BC, PER, 2)
        # noise_sp[64q+b, 16c+s] = ns[b, 16q+s, c]
        noise_sp = np.concatenate(
            [
                ns[:, 16 * q : 16 * q + 16, :]
                .transpose(0, 2, 1)
                .reshape(BC, 32)
                for q in range(2)
            ],
            axis=0,
        )
        pgs = gallery[pool_idx[rows]].astype(np.float32)   # [64, 160, 2]
        pool_gps = np.concatenate(
            [pgs[:, :80, :].reshape(BC, 160), pgs[:, 80:, :].reshape(BC, 160)],
            axis=0,
        )
        rf = rank_fix[rows]                                # [64, 160]
        sl = slot_full[rows]                               # [64, 32]
        in_maps.append(
            {
                "imgs": imgs,
                "w_img": w_img,
                "w1": w1,
                "b1r": b1r,
                "w2": w2,
                "b2r": b2r,
                "freqs": freqs,
                "lgs128": lgs128,
                "pool_gps": np.ascontiguousarray(pool_gps),
                "pool_lat": np.ascontiguousarray(np.tile(pgs[:, :, 0], (2, 1))),
                "pool_lon": np.ascontiguousarray(np.tile(pgs[:, :, 1], (2, 1))),
                "rank_fixd": np.ascontiguousarray(np.tile(rf, (2, 1))),
                "gps_dup": np.ascontiguousarray(np.tile(gps[rows], (2, 1))),
                "gpst_loc": np.ascontiguousarray(gps[rows].T),
                "slot_sp": np.ascontiguousarray(
                    np.concatenate([sl[:, :16], sl[:, 16:]], axis=0)
                ),
                "noise_sp": np.ascontiguousarray(noise_sp),
                "diagmask": dm,
                "bexp": bexp,
            }
        )
    return in_maps


def kernel(**inputs):
    nc = _get_program()
    in_maps = make_in_maps(inputs)
    res = run_bass_kernel_spmd(nc, in_maps, list(range(N_CORES)))
    se = np.zeros((1, B), np.float64)
    dg = np.zeros((N_CORES, BC), np.float64)
    for c in range(N_CORES):
        se += res.results[c]["se_part"]
        dg[c, :] = res.results[c]["diag_part"][0]
    loss = -np.mean(dg.reshape(-1) - np.log(se.reshape(-1)))
    return np.float32(loss)

